# revision 21
# baseline (speedup 1.0000x reference)
"""Multi-head causal attention (B=2, S=4096, D=1024, H=16) on 8 TRN2 NeuronCores.

Sharding: head-parallel. Core c computes heads 2c, 2c+1 (128 of the 1024
projection columns) for both batches:
  - QKV column-parallel: each core gets Wq/Wk/Wv[:, c*128:(c+1)*128]
  - attention for its 2 heads over all tokens (causal, full score rows,
    streamed in 512-query chunks, keys on PSUM partitions)
  - out-proj row-parallel: partial_out = ctx_c @ Wo[c*128:(c+1)*128, :]
  - host sums the 8 partials and adds bo.

x is transposed on the host (xT = x.reshape(T, D).T) because every matmul
on the PE contracts over the partition axis; this avoids all on-chip
transposes of x.

Pipeline per (kt = key tile, chunk = 512 queries):
  scores:  PSUM [128 keys, 2x512 queries (both heads)]  (PE)
  exp:     ACT psum -> SBUF ex bf16 [128, 1024]
  mask:    DVE mul on the diagonal block
  ctx:     ex tile is the STATIONARY operand: out[128 queries, 65] +=
           exT @ [v | ones]; the ones column of vA makes the softmax
           denominator land in out column 64 (per-query = per-partition).
           Moving operand is only 65 columns -> half the PE time of the
           v-stationary form.
  norm:    DVE reciprocal_approx_fast on the 4 denominators, then Pool
           tensor_scalar multiplies ctx by 1/den (per-partition scalar).
  cT:      PE transpose of the normalized [128 tok, 128 dims] tile
           (both heads at once) back to dims-major for the out-proj.
  outproj: PE matmul cT-tile @ Wo chunk; evac split DVE/Pool; DMA out.
"""

from contextlib import ExitStack

import numpy as np

import concourse.bass as bass
import concourse.tile as tile
from concourse import bacc, mybir
from concourse.bass_utils import run_bass_kernel_spmd

F32 = mybir.dt.float32
BF16 = mybir.dt.bfloat16
FP8 = mybir.dt.float8e4
P = 128
AF = mybir.ActivationFunctionType

N_CORES = 8
B_FULL, S_FULL, D_FULL, H_FULL = 2, 4096, 1024, 16
DH = 64
CW = 128  # projection columns per core (2 heads * 64)


def build_program(S=S_FULL, B=B_FULL, D=D_FULL):
    """Build the per-core Bass program (same program on all 8 cores)."""
    T = B * S
    KC = D // P            # contraction chunks for the projections
    IC = min(512, S)       # query-chunk width (paired-head layout)
    NQT = IC // P          # query tiles (128) per chunk
    NJ = S // P            # key tiles per batch
    NIC = S // IC          # query chunks per batch
    WN = min(512, T)       # QKV token window

    nc = bacc.Bacc("TRN2", target_bir_lowering=False, debug=False,
                   num_devices=N_CORES)

    xT = nc.dram_tensor("xT", [D, T], BF16, kind="ExternalInput").ap()
    wq = nc.dram_tensor("wq", [P, D // P, CW], BF16, kind="ExternalInput").ap()
    wk = nc.dram_tensor("wk", [P, D // P, CW], BF16, kind="ExternalInput").ap()
    wv = nc.dram_tensor("wv", [P, D // P, CW], BF16, kind="ExternalInput").ap()
    wo = nc.dram_tensor("wo", [CW, D], BF16, kind="ExternalInput").ap()
    maskneg = nc.dram_tensor("maskneg", [P, P], BF16, kind="ExternalInput").ap()
    ident = nc.dram_tensor("ident", [P, P], BF16, kind="ExternalInput").ap()
    out = nc.dram_tensor("out", [T, D], BF16, kind="ExternalOutput").ap()

    with tile.TileContext(nc) as tc, ExitStack() as ctx:
        singles = ctx.enter_context(tc.tile_pool(name="singles", bufs=1))
        qT = singles.tile([P, T], BF16, name="qT")
        kT = singles.tile([P, T], BF16, name="kT")
        vA = singles.tile([P, B * NJ, 130], BF16, name="vA")
        cT = singles.tile([P, T], BF16, name="cT")
        wq_s = singles.tile([P, KC, CW], BF16, name="wq_s")
        wk_s = singles.tile([P, KC, CW], BF16, name="wk_s")
        wv_s = singles.tile([P, KC, CW], BF16, name="wv_s")
        wo_s = singles.tile([CW, D], BF16, name="wo_s")
        maskneg_s = singles.tile([P, P], BF16, name="maskneg_s")
        ident_s = singles.tile([P, P], BF16, name="ident_s")

        nc.sync.dma_start(out=wq_s, in_=wq)
        nc.sync.dma_start(out=wk_s, in_=wk)
        nc.sync.dma_start(out=wv_s, in_=wv)
        nc.vector.memset(vA[:, :, 64:65], 1.0)
        nc.vector.memset(vA[:, :, 129:130], 1.0)

        xw_pool = ctx.enter_context(tc.tile_pool(name="xw_pool", bufs=3))
        # PSUM budget (8 banks): sm 2 + sc 2x2 + ctx 2x1 = 8
        sm_ps = ctx.enter_context(
            tc.tile_pool(name="sm_ps", bufs=2, space=bass.MemorySpace.PSUM))
        sc_ps = ctx.enter_context(
            tc.tile_pool(name="sc_ps", bufs=2, space=bass.MemorySpace.PSUM))
        cx_ps = ctx.enter_context(
            tc.tile_pool(name="cx_ps", bufs=1, space=bass.MemorySpace.PSUM))
        exp_sb = ctx.enter_context(tc.tile_pool(name="exp_sb", bufs=5))
        inv_sb = ctx.enter_context(tc.tile_pool(name="inv_sb", bufs=2))
        cn_sb = ctx.enter_context(tc.tile_pool(name="cn_sb", bufs=6))
        ob_sb = ctx.enter_context(tc.tile_pool(name="ob_sb", bufs=2))

        # --- fine-grained PE filler work queue -------------------------
        # Projection windows and chunk tails are split into ~0.5-2us
        # pieces, drained one per key tile so the PE filler work overlaps
        # ACT's exp (the attention inner loop is ACT-bound).
        fill_q = []          # entries: (kind, key, fn); kind "dma"/"q"
        # keyed by window, "k"/"v" by global token tile, tails (None, 0)

        def push_window(w, first=False):
            cell = {}

            def p_first():
                if first:
                    nc.sync.dma_start(out=wo_s, in_=wo)
                    nc.sync.dma_start(out=maskneg_s, in_=maskneg)
                    nc.sync.dma_start(out=ident_s, in_=ident)
                xw = xw_pool.tile([P, KC, WN], BF16, name="xw", tag="xw")
                cell["xw"] = xw
                for kc in range(KC):
                    nc.sync.dma_start(
                        out=xw[:, kc, :],
                        in_=xT[kc * P:(kc + 1) * P, w * WN:(w + 1) * WN])

            def p_q(st):
                xw = cell["xw"]
                t0 = w * WN + st * P
                q_ps = sm_ps.tile([P, P], F32, name="q_ps", tag="sm")
                for kc in range(KC):
                    nc.tensor.matmul(q_ps, wq_s[:, kc, :],
                                     xw[:, kc, st * P:(st + 1) * P],
                                     start=(kc == 0), stop=(kc == KC - 1))
                nc.vector.tensor_copy(qT[:, t0:t0 + P], q_ps)

            def p_k(st):
                xw = cell["xw"]
                t0 = w * WN + st * P
                k_ps = sm_ps.tile([P, P], F32, name="k_ps", tag="sm")
                for kc in range(KC):
                    nc.tensor.matmul(k_ps, wk_s[:, kc, :],
                                     xw[:, kc, st * P:(st + 1) * P],
                                     start=(kc == 0), stop=(kc == KC - 1))
                nc.vector.tensor_copy(kT[:, t0:t0 + P], k_ps)

            def p_v(st):
                xw = cell["xw"]
                jt = (w * WN) // P + st  # global token tile
                vp = sm_ps.tile([P, CW], F32, name="vp", tag="sm")
                for kc in range(KC):
                    nc.tensor.matmul(vp, xw[:, kc, st * P:(st + 1) * P],
                                     wv_s[:, kc, :],
                                     start=(kc == 0), stop=(kc == KC - 1))
                dst = vA[:, jt, 0:129]
                dst = bass.AP(tensor=dst.tensor, offset=dst.offset,
                              ap=[dst.ap[0], [65, 2], [1, 64]])
                srcap = bass.AP(tensor=vp.tensor, offset=vp.offset,
                                ap=[vp.ap[0], [64, 2], [1, 64]])
                nc.vector.tensor_copy(dst, srcap)

            def fused(fns):
                def run():
                    for f in fns:
                        f()
                return run

            nst = WN // P
            t0 = (w * WN) // P
            fill_q.append(("dma", w, p_first))
            for st in range(nst):
                fill_q.append(("q", w, lambda st=st: p_q(st)))
            for st in range(nst):
                fill_q.append(("k", t0 + st, lambda st=st: p_k(st)))
                fill_q.append(("v", t0 + st, lambda st=st: p_v(st)))

        def _run(ent):
            kind, key, fn = ent
            if kind in ("q", "k", "v"):
                w = key if kind == "q" else (key * P) // WN
                for idx, e2 in enumerate(fill_q):
                    if e2[0] == "dma" and e2[1] == w:
                        fill_q.pop(idx)
                        e2[2]()
                        break
            fn()

        horizon = {"win": 0, "tile": 0}

        def drain_one():
            if not fill_q:
                return False
            pick = None
            for idx, ent in enumerate(fill_q):
                if ent[0] in ("dma", "q") and ent[1] <= horizon["win"]:
                    pick = idx
                    break
            if pick is None:
                for idx, ent in enumerate(fill_q):
                    if ent[0] in ("k", "v") and ent[1] <= horizon["tile"]:
                        pick = idx
                        break
            _run(fill_q.pop(pick if pick is not None else 0))
            return True

        def need(pred):
            while True:
                for idx, ent in enumerate(fill_q):
                    if pred(ent):
                        _run(fill_q.pop(idx))
                        break
                else:
                    return

        attn_pend = {"fns": []}

        def emit_attn_chunk(b, icn):
            gi0 = b * S + icn * IC  # global query start
            njt = (icn + 1) * NQT   # key tiles for this chunk
            # per-head ctx accumulators [128 queries, qt, 65]; stride 80
            # keeps each qt slice 64B-aligned in its bank
            cxs = [cx_ps.tile([P, NQT, 80], F32, name=f"cx{h}", tag=f"cx{h}")
                   for h in range(2)]
            invs = [inv_sb.tile([P, NQT], F32, name="inv", tag=f"inv{h}",
                                bufs=3) for h in range(2)]

            def make_tails(l, cn):
                cell = {}

                def tail_a():
                    s0 = gi0 + l * P
                    tr = sm_ps.tile([P, P], BF16, name="tr", tag="sm")
                    nc.tensor.transpose(tr, cn, ident_s)
                    nc.vector.tensor_copy(cT[:, s0:s0 + P], tr)
                    ob = ob_sb.tile([P, D], BF16, name="ob", tag="ob")
                    cell["ob"] = ob
                    op = sm_ps.tile([P, 512], F32, name="op", tag="sm")
                    nc.tensor.matmul(op, cT[:, s0:s0 + P], wo_s[:, 0:512],
                                     start=True, stop=True)
                    nc.vector.tensor_copy(ob[:, 0:512], op)

                def tail_b():
                    s0 = gi0 + l * P
                    ob = cell["ob"]
                    op = sm_ps.tile([P, 512], F32, name="op", tag="sm")
                    nc.tensor.matmul(op, cT[:, s0:s0 + P], wo_s[:, 512:1024],
                                     start=True, stop=True)
                    nc.vector.tensor_copy(ob[:, 512:1024], op)
                    nc.sync.dma_start(out=out[s0:s0 + P, :], in_=ob)
                return tail_a, tail_b

            def emit_ctx_and_norm(jt, ex):
                # ctx matmuls for key tile jt, all query tiles >= jt.
                # start=True zeroes the whole 2KB psum bank (the "zero
                # region"), so only the bank's FIRST matmul sets it; the
                # other qtiles' first writes overwrite their still
                # pending-zero bytes and later writes accumulate.
                for h in range(2):
                    for l in range(NQT):
                        qt_g = icn * NQT + l  # global query tile
                        if qt_g < jt:
                            continue
                        nc.tensor.matmul(
                            cxs[h][:, l, 0:65],
                            ex[:, h * IC + l * P:h * IC + (l + 1) * P],
                            vA[:, b * NJ + jt, h * 65:(h + 1) * 65],
                            start=(jt == 0 and l == 0), stop=(jt == qt_g),
                            skip_group_check=True)
                ld = jt - icn * NQT
                if ld >= 0:
                    # qtile ld just finished accumulating: normalize now so
                    # the ctx banks free early; transpose+outproj deferred
                    cn = cn_sb.tile([P, P], BF16, name="cn", tag="cn",
                                    bufs=10)
                    for h in range(2):
                        nc.vector.reciprocal_approx_fast(
                            invs[h][:, ld:ld + 1], cxs[h][:, ld, 64:65])
                        nc.vector.tensor_scalar_mul(
                            cn[:, h * 64:(h + 1) * 64],
                            cxs[h][:, ld, 0:64], invs[h][:, ld:ld + 1])
                    ta, tb = make_tails(ld, cn)
                    fill_q.append(("tail", 0, ta))
                    fill_q.append(("tail", 0, tb))

            for jt in range(njt):
                il0 = max(0, jt * P - icn * IC)
                gj0 = b * S + jt * P
                diag = jt * P >= icn * IC
                gt = gj0 // P
                need(lambda e: e[0] in ("k", "v") and e[1] <= gt)
                horizon["tile"] = gt + 4
                sc = sc_ps.tile([P, 2 * IC], F32, name="sc", tag="sc")
                for h in range(2):
                    hp = h * 64
                    nc.tensor.matmul(
                        sc[:, h * IC + il0:(h + 1) * IC],
                        kT[hp:hp + 64, gj0:gj0 + P],
                        qT[hp:hp + 64, gi0 + il0:gi0 + IC],
                        start=True, stop=not diag)
                    if diag:
                        # causal mask: add -30000 to the strict upper
                        # triangle of the diagonal block on the PE
                        nc.tensor.matmul(
                            sc[:, h * IC + il0:h * IC + il0 + P],
                            ident_s, maskneg_s,
                            start=False, stop=True)
                ex = exp_sb.tile([P, 2 * IC], BF16, name="ex", tag="ex")
                exin = bass.AP(tensor=sc.tensor, offset=sc.offset + il0,
                               ap=[sc.ap[0], [IC, 2], [1, IC - il0]])
                exout = bass.AP(tensor=ex.tensor, offset=ex.offset + il0,
                                ap=[ex.ap[0], [IC, 2], [1, IC - il0]])
                nc.scalar.activation(exout, exin, AF.Exp, scale=0.125)
                # lag-2 software pipeline ACROSS chunks: ctx of key tile
                # jt-2 issues after scores of jt, so its stationary ex has
                # been ready for two exp-latencies and the PE never stalls
                if len(attn_pend["fns"]) >= 2:
                    attn_pend["fns"].pop(0)()
                # drain fillers only on full-width kts: partial diagonal
                # kts have short exps and no PE slack to hide filler work
                if il0 == 0:
                    drain_one()
                    drain_one()
                    if njt >= 16:
                        drain_one()
                if len(fill_q) < 8:
                    push_upto(state["pushed"] + 1)
                attn_pend["fns"].append(
                    lambda jt=jt, ex=ex: emit_ctx_and_norm(jt, ex))

        # Emission: window w covers tokens [w*WN,(w+1)*WN); chunk (b,
        # icn) needs windows covering tokens < b*S + (icn+1)*IC pushed
        # and drained before its scores; two extra windows of lookahead
        # keep the DMA off the critical path.
        nwin = T // WN
        state = {"pushed": 0}

        def push_upto(upto):
            while state["pushed"] < min(upto, nwin):
                push_window(state["pushed"], first=(state["pushed"] == 0))
                state["pushed"] += 1

        for b in range(B):
            for icn in range(NIC):
                wq_win = (b * S + icn * IC) // WN
                push_upto(wq_win + 3)
                need(lambda e: e[0] in ("dma", "q") and e[1] <= wq_win)
                horizon["win"] = min(wq_win + 1, nwin - 1)
                if icn == NIC - 1:
                    horizon["win"] = min(((b + 1) * S) // WN, nwin - 1)
                emit_attn_chunk(b, icn)
        for fn in attn_pend["fns"]:
            fn()
        attn_pend["fns"] = []
        while drain_one():
            pass

    nc.compile()
    return nc


def _warrange(w, bf16):
    # [D, CW] -> [P, D//P, CW] contiguous (the SBUF layout, so the DMA is
    # a single contiguous copy instead of 256B strided pieces)
    D, CW_ = w.shape
    return np.ascontiguousarray(
        w.reshape(D // P, P, CW_).transpose(1, 0, 2)).astype(bf16)


def make_in_maps(x, Wq, Wk, Wv, Wo):
    import ml_dtypes
    bf16 = ml_dtypes.bfloat16
    B, S, D = x.shape
    xT = np.ascontiguousarray(x.reshape(B * S, D).T).astype(bf16)
    maskneg = np.tril(np.full((P, P), -30000.0, dtype=np.float32),
                      -1).astype(bf16)
    ident = np.eye(P, dtype=bf16)
    in_maps = []
    for c in range(N_CORES):
        cs = slice(c * CW, (c + 1) * CW)
        in_maps.append({
            "xT": xT,
            "wq": _warrange(Wq[:, cs], bf16),
            "wk": _warrange(Wk[:, cs], bf16),
            "wv": _warrange(Wv[:, cs], bf16),
            "wo": np.ascontiguousarray(Wo[cs, :]).astype(bf16),
            "maskneg": maskneg,
            "ident": ident,
        })
    return in_maps


_CACHED_NC = None


def kernel(x, Wq, Wk, Wv, Wo, bo, _trace=False):
    global _CACHED_NC
    x = np.asarray(x, dtype=np.float32)
    B, S, D = x.shape
    if _CACHED_NC is None:
        _CACHED_NC = build_program(S=S, B=B, D=D)
    nc = _CACHED_NC
    in_maps = make_in_maps(x, np.asarray(Wq), np.asarray(Wk),
                           np.asarray(Wv), np.asarray(Wo))
    res = None
    for attempt in range(3):
        try:
            res = run_bass_kernel_spmd(nc, in_maps, list(range(N_CORES)),
                                       trace=_trace)
            break
        except Exception:
            if attempt == 2:
                raise
    out = np.zeros((B * S, D), dtype=np.float32)
    for c in range(N_CORES):
        out += res.results[c]["out"].astype(np.float32)
    out += np.asarray(bo, dtype=np.float32)[None, :]
    if _trace:
        kernel._last_result = res
    return out.reshape(B, S, D)


# revision 22
# speedup vs baseline: 1.0457x; 1.0457x over previous
"""Multi-head causal attention (B=2, S=4096, D=1024, H=16) on 8 TRN2 NeuronCores.

Sharding: head-parallel. Core c computes heads 2c, 2c+1 (128 of the 1024
projection columns) for both batches:
  - QKV column-parallel: each core gets Wq/Wk/Wv[:, c*128:(c+1)*128]
  - attention for its 2 heads over all tokens (causal, full score rows,
    streamed in 512-query chunks, keys on PSUM partitions)
  - out-proj row-parallel: partial_out = ctx_c @ Wo[c*128:(c+1)*128, :]
  - host sums the 8 partials and adds bo.

x is transposed on the host (xT = x.reshape(T, D).T) because every matmul
on the PE contracts over the partition axis; this avoids all on-chip
transposes of x.

Pipeline per (kt = key tile, chunk = 512 queries):
  scores:  PSUM [128 keys, 2x512 queries (both heads)]  (PE)
  exp:     ACT psum -> SBUF ex bf16 [128, 1024]
  mask:    DVE mul on the diagonal block
  ctx:     ex tile is the STATIONARY operand: out[128 queries, 65] +=
           exT @ [v | ones]; the ones column of vA makes the softmax
           denominator land in out column 64 (per-query = per-partition).
           Moving operand is only 65 columns -> half the PE time of the
           v-stationary form.
  norm:    DVE reciprocal_approx_fast on the 4 denominators, then Pool
           tensor_scalar multiplies ctx by 1/den (per-partition scalar).
  cT:      PE transpose of the normalized [128 tok, 128 dims] tile
           (both heads at once) back to dims-major for the out-proj.
  outproj: PE matmul cT-tile @ Wo chunk; evac split DVE/Pool; DMA out.
"""

from contextlib import ExitStack

import numpy as np

import concourse.bass as bass
import concourse.tile as tile
from concourse import bacc, mybir
from concourse.bass_utils import run_bass_kernel_spmd

F32 = mybir.dt.float32
BF16 = mybir.dt.bfloat16
FP8 = mybir.dt.float8e4
P = 128
AF = mybir.ActivationFunctionType

N_CORES = 8
B_FULL, S_FULL, D_FULL, H_FULL = 2, 4096, 1024, 16
DH = 64
CW = 128  # projection columns per core (2 heads * 64)


def build_program(S=S_FULL, B=B_FULL, D=D_FULL):
    """Build the per-core Bass program (same program on all 8 cores)."""
    T = B * S
    KC = D // P            # contraction chunks for the projections
    IC = min(512, S)       # query-chunk width (paired-head layout)
    NQT = IC // P          # query tiles (128) per chunk
    NJ = S // P            # key tiles per batch
    NIC = S // IC          # query chunks per batch
    WN = min(512, T)       # QKV token window

    nc = bacc.Bacc("TRN2", target_bir_lowering=False, debug=False,
                   num_devices=N_CORES)

    xT = nc.dram_tensor("xT", [D, T], BF16, kind="ExternalInput").ap()
    wq = nc.dram_tensor("wq", [P, D // P, CW], BF16, kind="ExternalInput").ap()
    wk = nc.dram_tensor("wk", [P, D // P, CW], BF16, kind="ExternalInput").ap()
    wv = nc.dram_tensor("wv", [P, D // P, CW], BF16, kind="ExternalInput").ap()
    wo = nc.dram_tensor("wo", [CW, D], BF16, kind="ExternalInput").ap()
    maskneg = nc.dram_tensor("maskneg", [P, P], BF16, kind="ExternalInput").ap()
    ident = nc.dram_tensor("ident", [P, P], BF16, kind="ExternalInput").ap()
    out = nc.dram_tensor("out", [T, D], BF16, kind="ExternalOutput").ap()

    with tile.TileContext(nc) as tc, ExitStack() as ctx:
        singles = ctx.enter_context(tc.tile_pool(name="singles", bufs=1))
        qT = singles.tile([P, T], BF16, name="qT")
        kT = singles.tile([P, T], BF16, name="kT")
        vA = singles.tile([P, B * NJ, 130], BF16, name="vA")
        cT = singles.tile([P, T], BF16, name="cT")
        wq_s = singles.tile([P, KC, CW], BF16, name="wq_s")
        wk_s = singles.tile([P, KC, CW], BF16, name="wk_s")
        wv_s = singles.tile([P, KC, CW], BF16, name="wv_s")
        wo_s = singles.tile([CW, D], BF16, name="wo_s")
        maskneg_s = singles.tile([P, P], BF16, name="maskneg_s")
        ident_s = singles.tile([P, P], BF16, name="ident_s")

        nc.sync.dma_start(out=wq_s, in_=wq)
        nc.sync.dma_start(out=wk_s, in_=wk)
        nc.sync.dma_start(out=wv_s, in_=wv)
        nc.vector.memset(vA[:, :, 64:65], 1.0)
        nc.vector.memset(vA[:, :, 129:130], 1.0)

        xw_pool = ctx.enter_context(tc.tile_pool(name="xw_pool", bufs=3))
        # PSUM budget (8 banks): sm 2 + sc 2x2 + ctx 2x1 = 8
        sm_ps = ctx.enter_context(
            tc.tile_pool(name="sm_ps", bufs=2, space=bass.MemorySpace.PSUM))
        sc_ps = ctx.enter_context(
            tc.tile_pool(name="sc_ps", bufs=2, space=bass.MemorySpace.PSUM))
        cx_ps = ctx.enter_context(
            tc.tile_pool(name="cx_ps", bufs=1, space=bass.MemorySpace.PSUM))
        exp_sb = ctx.enter_context(tc.tile_pool(name="exp_sb", bufs=5))
        inv_sb = ctx.enter_context(tc.tile_pool(name="inv_sb", bufs=2))
        cn_sb = ctx.enter_context(tc.tile_pool(name="cn_sb", bufs=6))
        ob_sb = ctx.enter_context(tc.tile_pool(name="ob_sb", bufs=2))

        # --- fine-grained PE filler work queue -------------------------
        # Projection windows and chunk tails are split into ~0.5-2us
        # pieces, drained one per key tile so the PE filler work overlaps
        # ACT's exp (the attention inner loop is ACT-bound).
        fill_q = []          # entries: (kind, key, fn); kind "dma"/"q"
        # keyed by window, "k"/"v" by global token tile, tails (None, 0)

        def push_window(w, first=False):
            cell = {}

            def p_first():
                if first:
                    nc.sync.dma_start(out=wo_s, in_=wo)
                    nc.sync.dma_start(out=maskneg_s, in_=maskneg)
                    nc.sync.dma_start(out=ident_s, in_=ident)
                xw = xw_pool.tile([P, KC, WN], BF16, name="xw", tag="xw")
                cell["xw"] = xw
                for kc in range(KC):
                    nc.sync.dma_start(
                        out=xw[:, kc, :],
                        in_=xT[kc * P:(kc + 1) * P, w * WN:(w + 1) * WN])

            def p_q(st):
                xw = cell["xw"]
                t0 = w * WN + st * P
                q_ps = sm_ps.tile([P, P], F32, name="q_ps", tag="sm")
                for kc in range(KC):
                    nc.tensor.matmul(q_ps, wq_s[:, kc, :],
                                     xw[:, kc, st * P:(st + 1) * P],
                                     start=(kc == 0), stop=(kc == KC - 1))
                nc.vector.tensor_copy(qT[:, t0:t0 + P], q_ps)

            def p_k(st):
                xw = cell["xw"]
                t0 = w * WN + st * P
                k_ps = sm_ps.tile([P, P], F32, name="k_ps", tag="sm")
                for kc in range(KC):
                    nc.tensor.matmul(k_ps, wk_s[:, kc, :],
                                     xw[:, kc, st * P:(st + 1) * P],
                                     start=(kc == 0), stop=(kc == KC - 1))
                nc.vector.tensor_copy(kT[:, t0:t0 + P], k_ps)

            def p_v(st):
                xw = cell["xw"]
                jt = (w * WN) // P + st  # global token tile
                vp = sm_ps.tile([P, CW], F32, name="vp", tag="sm")
                for kc in range(KC):
                    nc.tensor.matmul(vp, xw[:, kc, st * P:(st + 1) * P],
                                     wv_s[:, kc, :],
                                     start=(kc == 0), stop=(kc == KC - 1))
                dst = vA[:, jt, 0:129]
                dst = bass.AP(tensor=dst.tensor, offset=dst.offset,
                              ap=[dst.ap[0], [65, 2], [1, 64]])
                srcap = bass.AP(tensor=vp.tensor, offset=vp.offset,
                                ap=[vp.ap[0], [64, 2], [1, 64]])
                nc.vector.tensor_copy(dst, srcap)

            def fused(fns):
                def run():
                    for f in fns:
                        f()
                return run

            nst = WN // P
            t0 = (w * WN) // P
            fill_q.append(("dma", w, p_first))
            for st in range(nst):
                fill_q.append(("q", w, lambda st=st: p_q(st)))
            for st in range(nst):
                fill_q.append(("k", t0 + st, lambda st=st: p_k(st)))
                fill_q.append(("v", t0 + st, lambda st=st: p_v(st)))

        def _run(ent):
            kind, key, fn = ent
            if kind in ("q", "k", "v"):
                w = key if kind == "q" else (key * P) // WN
                for idx, e2 in enumerate(fill_q):
                    if e2[0] == "dma" and e2[1] == w:
                        fill_q.pop(idx)
                        e2[2]()
                        break
            fn()

        horizon = {"win": 0, "tile": 0}

        def drain_one():
            if not fill_q:
                return False
            pick = None
            for idx, ent in enumerate(fill_q):
                if ent[0] in ("dma", "q") and ent[1] <= horizon["win"]:
                    pick = idx
                    break
            if pick is None:
                for idx, ent in enumerate(fill_q):
                    if ent[0] in ("k", "v") and ent[1] <= horizon["tile"]:
                        pick = idx
                        break
            _run(fill_q.pop(pick if pick is not None else 0))
            return True

        def need(pred):
            while True:
                for idx, ent in enumerate(fill_q):
                    if pred(ent):
                        _run(fill_q.pop(idx))
                        break
                else:
                    return

        attn_pend = {"fns": []}

        def emit_attn_chunk(b, icn):
            gi0 = b * S + icn * IC  # global query start
            njt = (icn + 1) * NQT   # key tiles for this chunk
            # per-head ctx accumulators [128 queries, qt, 65]; stride 80
            # keeps each qt slice 64B-aligned in its bank
            cxs = [cx_ps.tile([P, NQT, 80], F32, name=f"cx{h}", tag=f"cx{h}")
                   for h in range(2)]
            invs = [inv_sb.tile([P, NQT], F32, name="inv", tag=f"inv{h}",
                                bufs=3) for h in range(2)]

            def make_tails(l, cn):
                cell = {}

                def tail_a():
                    s0 = gi0 + l * P
                    tr = sm_ps.tile([P, P], BF16, name="tr", tag="sm")
                    nc.tensor.transpose(tr, cn, ident_s)
                    nc.vector.tensor_copy(cT[:, s0:s0 + P], tr)
                    ob = ob_sb.tile([P, D], BF16, name="ob", tag="ob")
                    cell["ob"] = ob
                    op = sm_ps.tile([P, 512], F32, name="op", tag="sm")
                    nc.tensor.matmul(op, cT[:, s0:s0 + P], wo_s[:, 0:512],
                                     start=True, stop=True)
                    nc.vector.tensor_copy(ob[:, 0:512], op)

                def tail_b():
                    s0 = gi0 + l * P
                    ob = cell["ob"]
                    op = sm_ps.tile([P, 512], F32, name="op", tag="sm")
                    nc.tensor.matmul(op, cT[:, s0:s0 + P], wo_s[:, 512:1024],
                                     start=True, stop=True)
                    nc.vector.tensor_copy(ob[:, 512:1024], op)
                    nc.sync.dma_start(out=out[s0:s0 + P, :], in_=ob)
                return tail_a, tail_b

            def emit_ctx_and_norm(jt, ex):
                # ctx matmuls for key tile jt, all query tiles >= jt.
                # start=True zeroes the whole 2KB psum bank (the "zero
                # region"), so only the bank's FIRST matmul sets it; the
                # other qtiles' first writes overwrite their still
                # pending-zero bytes and later writes accumulate.
                for h in range(2):
                    for l in range(NQT):
                        qt_g = icn * NQT + l  # global query tile
                        if qt_g < jt:
                            continue
                        nc.tensor.matmul(
                            cxs[h][:, l, 0:65],
                            ex[:, h * IC + l * P:h * IC + (l + 1) * P],
                            vA[:, b * NJ + jt, h * 65:(h + 1) * 65],
                            start=(jt == 0 and l == 0), stop=(jt == qt_g),
                            skip_group_check=True)
                ld = jt - icn * NQT
                if ld >= 0:
                    # qtile ld just finished accumulating: normalize now so
                    # the ctx banks free early; transpose+outproj deferred
                    cn = cn_sb.tile([P, P], BF16, name="cn", tag="cn",
                                    bufs=10)
                    for h in range(2):
                        nc.vector.reciprocal_approx_fast(
                            invs[h][:, ld:ld + 1], cxs[h][:, ld, 64:65])
                        nc.vector.tensor_scalar_mul(
                            cn[:, h * 64:(h + 1) * 64],
                            cxs[h][:, ld, 0:64], invs[h][:, ld:ld + 1])
                    ta, tb = make_tails(ld, cn)
                    fill_q.append(("tail", 0, ta))
                    fill_q.append(("tail", 0, tb))

            for jt in range(njt):
                il0 = max(0, jt * P - icn * IC)
                gj0 = b * S + jt * P
                diag = jt * P >= icn * IC
                gt = gj0 // P
                need(lambda e: e[0] in ("k", "v") and e[1] <= gt)
                horizon["tile"] = gt + 4
                sc = sc_ps.tile([P, 2 * IC], F32, name="sc", tag="sc")
                for h in range(2):
                    hp = h * 64
                    nc.tensor.matmul(
                        sc[:, h * IC + il0:(h + 1) * IC],
                        kT[hp:hp + 64, gj0:gj0 + P],
                        qT[hp:hp + 64, gi0 + il0:gi0 + IC],
                        start=True, stop=not diag)
                    if diag:
                        # causal mask: add -30000 to the strict upper
                        # triangle of the diagonal block on the PE
                        nc.tensor.matmul(
                            sc[:, h * IC + il0:h * IC + il0 + P],
                            ident_s, maskneg_s,
                            start=False, stop=True)
                ex = exp_sb.tile([P, 2 * IC], BF16, name="ex", tag="ex")
                exin = bass.AP(tensor=sc.tensor, offset=sc.offset + il0,
                               ap=[sc.ap[0], [IC, 2], [1, IC - il0]])
                exout = bass.AP(tensor=ex.tensor, offset=ex.offset + il0,
                                ap=[ex.ap[0], [IC, 2], [1, IC - il0]])
                nc.scalar.activation(exout, exin, AF.Exp, scale=0.125)
                # lag-2 software pipeline ACROSS chunks: ctx of key tile
                # jt-2 issues after scores of jt, so its stationary ex has
                # been ready for two exp-latencies and the PE never stalls
                if len(attn_pend["fns"]) >= 2:
                    attn_pend["fns"].pop(0)()
                # drain fillers only on full-width kts: partial diagonal
                # kts have short exps and no PE slack to hide filler work
                if il0 == 0:
                    drain_one()
                    if njt >= 16:
                        drain_one()
                if len(fill_q) < 8:
                    push_upto(state["pushed"] + 1)
                attn_pend["fns"].append(
                    lambda jt=jt, ex=ex: emit_ctx_and_norm(jt, ex))

        # Emission: window w covers tokens [w*WN,(w+1)*WN); chunk (b,
        # icn) needs windows covering tokens < b*S + (icn+1)*IC pushed
        # and drained before its scores; two extra windows of lookahead
        # keep the DMA off the critical path.
        nwin = T // WN
        state = {"pushed": 0}

        def push_upto(upto):
            while state["pushed"] < min(upto, nwin):
                push_window(state["pushed"], first=(state["pushed"] == 0))
                state["pushed"] += 1

        for b in range(B):
            for icn in range(NIC):
                wq_win = (b * S + icn * IC) // WN
                push_upto(wq_win + 3)
                need(lambda e: e[0] in ("dma", "q") and e[1] <= wq_win)
                horizon["win"] = min(wq_win + 1, nwin - 1)
                if icn == NIC - 1:
                    horizon["win"] = min(((b + 1) * S) // WN, nwin - 1)
                emit_attn_chunk(b, icn)
        for fn in attn_pend["fns"]:
            fn()
        attn_pend["fns"] = []
        while drain_one():
            pass

    nc.compile()
    return nc


def _warrange(w, bf16):
    # [D, CW] -> [P, D//P, CW] contiguous (the SBUF layout, so the DMA is
    # a single contiguous copy instead of 256B strided pieces)
    D, CW_ = w.shape
    return np.ascontiguousarray(
        w.reshape(D // P, P, CW_).transpose(1, 0, 2)).astype(bf16)


def make_in_maps(x, Wq, Wk, Wv, Wo):
    import ml_dtypes
    bf16 = ml_dtypes.bfloat16
    B, S, D = x.shape
    xT = np.ascontiguousarray(x.reshape(B * S, D).T).astype(bf16)
    maskneg = np.tril(np.full((P, P), -30000.0, dtype=np.float32),
                      -1).astype(bf16)
    ident = np.eye(P, dtype=bf16)
    in_maps = []
    for c in range(N_CORES):
        cs = slice(c * CW, (c + 1) * CW)
        in_maps.append({
            "xT": xT,
            "wq": _warrange(Wq[:, cs], bf16),
            "wk": _warrange(Wk[:, cs], bf16),
            "wv": _warrange(Wv[:, cs], bf16),
            "wo": np.ascontiguousarray(Wo[cs, :]).astype(bf16),
            "maskneg": maskneg,
            "ident": ident,
        })
    return in_maps


_CACHED_NC = None


def kernel(x, Wq, Wk, Wv, Wo, bo, _trace=False):
    global _CACHED_NC
    x = np.asarray(x, dtype=np.float32)
    B, S, D = x.shape
    if _CACHED_NC is None:
        _CACHED_NC = build_program(S=S, B=B, D=D)
    nc = _CACHED_NC
    in_maps = make_in_maps(x, np.asarray(Wq), np.asarray(Wk),
                           np.asarray(Wv), np.asarray(Wo))
    res = None
    for attempt in range(3):
        try:
            res = run_bass_kernel_spmd(nc, in_maps, list(range(N_CORES)),
                                       trace=_trace)
            break
        except Exception:
            if attempt == 2:
                raise
    out = np.zeros((B * S, D), dtype=np.float32)
    for c in range(N_CORES):
        out += res.results[c]["out"].astype(np.float32)
    out += np.asarray(bo, dtype=np.float32)[None, :]
    if _trace:
        kernel._last_result = res
    return out.reshape(B, S, D)


# revision 31
# speedup vs baseline: 1.0932x; 1.0454x over previous
"""Multi-head causal attention (B=2, S=4096, D=1024, H=16) on 8 TRN2 NeuronCores.

Sharding: head-parallel. Core c computes heads 2c, 2c+1 (128 of the 1024
projection columns) for both batches:
  - QKV column-parallel: each core gets Wq/Wk/Wv[:, c*128:(c+1)*128]
  - attention for its 2 heads over all tokens (causal, full score rows,
    streamed in 512-query chunks, keys on PSUM partitions)
  - out-proj row-parallel: partial_out = ctx_c @ Wo[c*128:(c+1)*128, :]
  - host sums the 8 partials and adds bo.

x is transposed on the host (xT = x.reshape(T, D).T) because every matmul
on the PE contracts over the partition axis.

Attention inner loop, per key tile (kt) of each 512-query chunk:
  scores:  PE -> PSUM [128 keys, 2x512 queries (both heads)]; the causal
           mask is a second accumulating matmul adding -30000 to the
           diagonal block's upper triangle (identity stationary).
  exp:     one ACT instruction per kt (strided AP covers both heads'
           valid ranges) -> SBUF ex bf16. ACT is the bottleneck engine
           (~280us of exp); everything else is scheduled around it.
  ctx:     ex is the STATIONARY operand: out[128 queries, 65] +=
           exT @ [v | ones]; the ones column of vA makes the softmax
           denominator land in column 64 (per-query = per-partition).
           Moving operand is 65 columns -> half the PE cost of the
           v-stationary form. All query tiles of a chunk accumulate in
           one PSUM bank per head; only the bank's first matmul carries
           start=True (start zeroes the whole 2KB zero region).
  norm:    as each query tile's accumulation stops (at its diagonal kt),
           DVE reciprocal_approx_fast on the denominator + tensor_scalar
           multiply into a bf16 [128 tok, 128] staging tile.
  cT:      PE transpose (both heads at once) back to dims-major.
  outproj: PE matmul cT-tile @ Wo; DVE evacuation; DMA out.

Scheduling: ctx runs at lag-2 behind scores so its stationary ex is
always ready. Projection windows are split into ~430ns atoms (per-128-
token q/k/v pieces) and chunk tails into two atoms; a priority work
queue drains them only on full-width kts (partial diagonal kts have no
PE slack), preferring pieces needed soonest: next chunk's q pieces,
then near-term k/v pieces, then FIFO.
"""

from contextlib import ExitStack

import numpy as np

import concourse.bass as bass
import concourse.tile as tile
from concourse import bacc, mybir
from concourse.bass_utils import run_bass_kernel_spmd

F32 = mybir.dt.float32
BF16 = mybir.dt.bfloat16
FP8 = mybir.dt.float8e4
P = 128
AF = mybir.ActivationFunctionType

N_CORES = 8
B_FULL, S_FULL, D_FULL, H_FULL = 2, 4096, 1024, 16
DH = 64
CW = 128  # projection columns per core (2 heads * 64)


def build_program(S=S_FULL, B=B_FULL, D=D_FULL):
    """Build the per-core Bass program (same program on all 8 cores)."""
    T = B * S
    KC = D // P            # contraction chunks for the projections
    IC = min(512, S)       # query-chunk width (paired-head layout)
    NQT = IC // P          # query tiles (128) per chunk
    NJ = S // P            # key tiles per batch
    NIC = S // IC          # query chunks per batch
    WN = min(512, T)       # QKV token window

    nc = bacc.Bacc("TRN2", target_bir_lowering=False, debug=False,
                   num_devices=N_CORES)

    xT = nc.dram_tensor("xT", [D, T], BF16, kind="ExternalInput").ap()
    wq = nc.dram_tensor("wq", [P, D // P, CW], BF16, kind="ExternalInput").ap()
    wk = nc.dram_tensor("wk", [P, D // P, CW], BF16, kind="ExternalInput").ap()
    wv = nc.dram_tensor("wv", [P, D // P, CW], BF16, kind="ExternalInput").ap()
    wo = nc.dram_tensor("wo", [CW, D], BF16, kind="ExternalInput").ap()
    maskneg = nc.dram_tensor("maskneg", [P, P], BF16, kind="ExternalInput").ap()
    ident = nc.dram_tensor("ident", [P, P], BF16, kind="ExternalInput").ap()
    out = nc.dram_tensor("out", [T, D], BF16, kind="ExternalOutput").ap()

    with tile.TileContext(nc) as tc, ExitStack() as ctx:
        singles = ctx.enter_context(tc.tile_pool(name="singles", bufs=1))
        qT = singles.tile([P, T], BF16, name="qT")
        kT = singles.tile([P, T], BF16, name="kT")
        vA = singles.tile([P, B * NJ, 130], BF16, name="vA")
        cT = singles.tile([P, T], BF16, name="cT")
        wq_s = singles.tile([P, KC, CW], BF16, name="wq_s")
        wk_s = singles.tile([P, KC, CW], BF16, name="wk_s")
        wv_s = singles.tile([P, KC, CW], BF16, name="wv_s")
        wo_s = singles.tile([CW, D], BF16, name="wo_s")
        maskneg_s = singles.tile([P, P], BF16, name="maskneg_s")
        ident_s = singles.tile([P, P], BF16, name="ident_s")

        nc.sync.dma_start(out=wq_s, in_=wq)
        nc.sync.dma_start(out=wk_s, in_=wk)
        nc.sync.dma_start(out=wv_s, in_=wv)
        nc.vector.memset(vA[:, :, 64:65], 1.0)
        nc.vector.memset(vA[:, :, 129:130], 1.0)

        xw_pool = ctx.enter_context(tc.tile_pool(name="xw_pool", bufs=3))
        # PSUM budget (8 banks): sm 2 + sc 2x2 + ctx 2x1 = 8
        sm_ps = ctx.enter_context(
            tc.tile_pool(name="sm_ps", bufs=2, space=bass.MemorySpace.PSUM))
        sc_ps = ctx.enter_context(
            tc.tile_pool(name="sc_ps", bufs=2, space=bass.MemorySpace.PSUM))
        cx_ps = ctx.enter_context(
            tc.tile_pool(name="cx_ps", bufs=1, space=bass.MemorySpace.PSUM))
        exp_sb = ctx.enter_context(tc.tile_pool(name="exp_sb", bufs=5))
        inv_sb = ctx.enter_context(tc.tile_pool(name="inv_sb", bufs=2))
        cn_sb = ctx.enter_context(tc.tile_pool(name="cn_sb", bufs=6))
        ob_sb = ctx.enter_context(tc.tile_pool(name="ob_sb", bufs=2))

        # --- fine-grained PE filler work queue -------------------------
        # Projection windows and chunk tails are split into ~0.5-2us
        # pieces, drained one per key tile so the PE filler work overlaps
        # ACT's exp (the attention inner loop is ACT-bound).
        fill_q = []          # entries: (kind, key, fn); kind "dma"/"q"
        # keyed by window, "k"/"v" by global token tile, tails (None, 0)

        def push_window(w, first=False):
            cell = {}

            def p_first():
                if first:
                    nc.sync.dma_start(out=wo_s, in_=wo)
                    nc.sync.dma_start(out=maskneg_s, in_=maskneg)
                    nc.sync.dma_start(out=ident_s, in_=ident)
                xw = xw_pool.tile([P, KC, WN], BF16, name="xw", tag="xw")
                cell["xw"] = xw
                for kc in range(KC):
                    nc.sync.dma_start(
                        out=xw[:, kc, :],
                        in_=xT[kc * P:(kc + 1) * P, w * WN:(w + 1) * WN])

            def p_q(st):
                xw = cell["xw"]
                t0 = w * WN + st * P
                q_ps = sm_ps.tile([P, P], F32, name="q_ps", tag="sm")
                for kc in range(KC):
                    nc.tensor.matmul(q_ps, wq_s[:, kc, :],
                                     xw[:, kc, st * P:(st + 1) * P],
                                     start=(kc == 0), stop=(kc == KC - 1))
                nc.vector.tensor_copy(qT[:, t0:t0 + P], q_ps)

            def p_k(st):
                xw = cell["xw"]
                t0 = w * WN + st * P
                k_ps = sm_ps.tile([P, P], F32, name="k_ps", tag="sm")
                for kc in range(KC):
                    nc.tensor.matmul(k_ps, wk_s[:, kc, :],
                                     xw[:, kc, st * P:(st + 1) * P],
                                     start=(kc == 0), stop=(kc == KC - 1))
                nc.vector.tensor_copy(kT[:, t0:t0 + P], k_ps)

            def p_v(st):
                xw = cell["xw"]
                jt = (w * WN) // P + st  # global token tile
                vp = sm_ps.tile([P, CW], F32, name="vp", tag="sm")
                for kc in range(KC):
                    nc.tensor.matmul(vp, xw[:, kc, st * P:(st + 1) * P],
                                     wv_s[:, kc, :],
                                     start=(kc == 0), stop=(kc == KC - 1))
                dst = vA[:, jt, 0:129]
                dst = bass.AP(tensor=dst.tensor, offset=dst.offset,
                              ap=[dst.ap[0], [65, 2], [1, 64]])
                srcap = bass.AP(tensor=vp.tensor, offset=vp.offset,
                                ap=[vp.ap[0], [64, 2], [1, 64]])
                nc.vector.tensor_copy(dst, srcap)

            def fused(fns):
                def run():
                    for f in fns:
                        f()
                return run

            nst = WN // P
            t0 = (w * WN) // P
            fill_q.append(("dma", w, p_first))
            for st in range(nst):
                fill_q.append(("q", w, lambda st=st: p_q(st)))
            for st in range(nst):
                fill_q.append(("k", t0 + st, lambda st=st: p_k(st)))
                fill_q.append(("v", t0 + st, lambda st=st: p_v(st)))

        def _run(ent):
            kind, key, fn = ent
            if kind in ("q", "k", "v"):
                w = key if kind == "q" else (key * P) // WN
                for idx, e2 in enumerate(fill_q):
                    if e2[0] == "dma" and e2[1] == w:
                        fill_q.pop(idx)
                        e2[2]()
                        break
            fn()

        horizon = {"win": 0, "tile": 0}

        def drain_one():
            if not fill_q:
                return False
            pick = None
            for idx, ent in enumerate(fill_q):
                if ent[0] in ("dma", "q") and ent[1] <= horizon["win"]:
                    pick = idx
                    break
            if pick is None:
                for idx, ent in enumerate(fill_q):
                    if ent[0] in ("k", "v") and ent[1] <= horizon["tile"]:
                        pick = idx
                        break
            _run(fill_q.pop(pick if pick is not None else 0))
            return True

        def need(pred):
            while True:
                for idx, ent in enumerate(fill_q):
                    if pred(ent):
                        _run(fill_q.pop(idx))
                        break
                else:
                    return

        attn_pend = {"fns": []}

        def emit_attn_chunk(b, icn):
            gi0 = b * S + icn * IC  # global query start
            njt = (icn + 1) * NQT   # key tiles for this chunk
            # per-head ctx accumulators [128 queries, qt, 65]; stride 80
            # keeps each qt slice 64B-aligned in its bank
            cxs = [cx_ps.tile([P, NQT, 80], F32, name=f"cx{h}", tag=f"cx{h}")
                   for h in range(2)]
            invs = [inv_sb.tile([P, NQT], F32, name="inv", tag=f"inv{h}",
                                bufs=3) for h in range(2)]

            def make_tails(l, cn):
                cell = {}

                def tail_a():
                    s0 = gi0 + l * P
                    tr = sm_ps.tile([P, P], BF16, name="tr", tag="sm")
                    nc.tensor.transpose(tr, cn, ident_s)
                    nc.vector.tensor_copy(cT[:, s0:s0 + P], tr)
                    ob = ob_sb.tile([P, D], BF16, name="ob", tag="ob")
                    cell["ob"] = ob
                    op = sm_ps.tile([P, 512], F32, name="op", tag="sm")
                    nc.tensor.matmul(op, cT[:, s0:s0 + P], wo_s[:, 0:512],
                                     start=True, stop=True)
                    nc.vector.tensor_copy(ob[:, 0:512], op)

                def tail_b():
                    s0 = gi0 + l * P
                    ob = cell["ob"]
                    op = sm_ps.tile([P, 512], F32, name="op", tag="sm")
                    nc.tensor.matmul(op, cT[:, s0:s0 + P], wo_s[:, 512:1024],
                                     start=True, stop=True)
                    nc.vector.tensor_copy(ob[:, 512:1024], op)
                    nc.sync.dma_start(out=out[s0:s0 + P, :], in_=ob)
                return tail_a, tail_b

            def emit_ctx_and_norm(jt, ex):
                # ctx matmuls for key tile jt, all query tiles >= jt.
                # start=True zeroes the whole 2KB psum bank (the "zero
                # region"), so only the bank's FIRST matmul sets it; the
                # other qtiles' first writes overwrite their still
                # pending-zero bytes and later writes accumulate.
                for h in range(2):
                    for l in range(NQT):
                        qt_g = icn * NQT + l  # global query tile
                        if qt_g < jt:
                            continue
                        nc.tensor.matmul(
                            cxs[h][:, l, 0:65],
                            ex[:, h * IC + l * P:h * IC + (l + 1) * P],
                            vA[:, b * NJ + jt, h * 65:(h + 1) * 65],
                            start=(jt == 0 and l == 0), stop=(jt == qt_g),
                            skip_group_check=True)
                ld = jt - icn * NQT
                if ld >= 0:
                    # qtile ld just finished accumulating: normalize now so
                    # the ctx banks free early; transpose+outproj deferred
                    cn = cn_sb.tile([P, P], BF16, name="cn", tag="cn",
                                    bufs=10)
                    for h in range(2):
                        nc.vector.reciprocal_approx_fast(
                            invs[h][:, ld:ld + 1], cxs[h][:, ld, 64:65])
                        nc.vector.tensor_scalar_mul(
                            cn[:, h * 64:(h + 1) * 64],
                            cxs[h][:, ld, 0:64], invs[h][:, ld:ld + 1])
                    ta, tb = make_tails(ld, cn)
                    fill_q.append(("tail", 0, ta))
                    fill_q.append(("tail", 0, tb))

            for jt in range(njt):
                il0 = max(0, jt * P - icn * IC)
                gj0 = b * S + jt * P
                diag = jt * P >= icn * IC
                gt = gj0 // P
                need(lambda e: e[0] in ("k", "v") and e[1] <= gt)
                horizon["tile"] = gt + 4
                sc = sc_ps.tile([P, 2 * IC], F32, name="sc", tag="sc")
                for h in range(2):
                    hp = h * 64
                    nc.tensor.matmul(
                        sc[:, h * IC + il0:(h + 1) * IC],
                        kT[hp:hp + 64, gj0:gj0 + P],
                        qT[hp:hp + 64, gi0 + il0:gi0 + IC],
                        start=True, stop=not diag)
                    if diag:
                        # causal mask: add -30000 to the strict upper
                        # triangle of the diagonal block on the PE
                        nc.tensor.matmul(
                            sc[:, h * IC + il0:h * IC + il0 + P],
                            ident_s, maskneg_s,
                            start=False, stop=True)
                ex = exp_sb.tile([P, 2 * IC], BF16, name="ex", tag="ex")
                exin = bass.AP(tensor=sc.tensor, offset=sc.offset + il0,
                               ap=[sc.ap[0], [IC, 2], [1, IC - il0]])
                exout = bass.AP(tensor=ex.tensor, offset=ex.offset + il0,
                                ap=[ex.ap[0], [IC, 2], [1, IC - il0]])
                nc.scalar.activation(exout, exin, AF.Exp, scale=0.125)
                # lag-2 software pipeline ACROSS chunks: ctx of key tile
                # jt-2 issues after scores of jt, so its stationary ex has
                # been ready for two exp-latencies and the PE never stalls
                if len(attn_pend["fns"]) >= 2:
                    attn_pend["fns"].pop(0)()
                # drain fillers only on full-width kts: partial diagonal
                # kts have short exps and no PE slack to hide filler work
                if il0 == 0:
                    drain_one()
                    if njt >= 16 and len(fill_q) > 40:
                        drain_one()
                if len(fill_q) < 16:
                    push_upto(state["pushed"] + 1)
                attn_pend["fns"].append(
                    lambda jt=jt, ex=ex: emit_ctx_and_norm(jt, ex))

        # Emission: window w covers tokens [w*WN,(w+1)*WN); chunk (b,
        # icn) needs windows covering tokens < b*S + (icn+1)*IC pushed
        # and drained before its scores; two extra windows of lookahead
        # keep the DMA off the critical path.
        nwin = T // WN
        state = {"pushed": 0}

        def push_upto(upto):
            while state["pushed"] < min(upto, nwin):
                push_window(state["pushed"], first=(state["pushed"] == 0))
                state["pushed"] += 1

        for b in range(B):
            for icn in range(NIC):
                wq_win = (b * S + icn * IC) // WN
                push_upto(wq_win + 3)
                need(lambda e: e[0] in ("dma", "q") and e[1] <= wq_win)
                horizon["win"] = min(wq_win + 1, nwin - 1)
                if icn == NIC - 1:
                    horizon["win"] = min(((b + 1) * S) // WN, nwin - 1)
                emit_attn_chunk(b, icn)
        for fn in attn_pend["fns"]:
            fn()
        attn_pend["fns"] = []
        while drain_one():
            pass

    nc.compile()
    return nc


def _warrange(w, bf16):
    # [D, CW] -> [P, D//P, CW] contiguous (the SBUF layout, so the DMA is
    # a single contiguous copy instead of 256B strided pieces)
    D, CW_ = w.shape
    return np.ascontiguousarray(
        w.reshape(D // P, P, CW_).transpose(1, 0, 2)).astype(bf16)


def make_in_maps(x, Wq, Wk, Wv, Wo):
    import ml_dtypes
    bf16 = ml_dtypes.bfloat16
    B, S, D = x.shape
    xT = np.ascontiguousarray(x.reshape(B * S, D).T).astype(bf16)
    maskneg = np.tril(np.full((P, P), -30000.0, dtype=np.float32),
                      -1).astype(bf16)
    ident = np.eye(P, dtype=bf16)
    in_maps = []
    for c in range(N_CORES):
        cs = slice(c * CW, (c + 1) * CW)
        in_maps.append({
            "xT": xT,
            "wq": _warrange(Wq[:, cs], bf16),
            "wk": _warrange(Wk[:, cs], bf16),
            "wv": _warrange(Wv[:, cs], bf16),
            "wo": np.ascontiguousarray(Wo[cs, :]).astype(bf16),
            "maskneg": maskneg,
            "ident": ident,
        })
    return in_maps


_CACHED_NC = None


def kernel(x, Wq, Wk, Wv, Wo, bo, _trace=False):
    global _CACHED_NC
    x = np.asarray(x, dtype=np.float32)
    B, S, D = x.shape
    if _CACHED_NC is None:
        _CACHED_NC = build_program(S=S, B=B, D=D)
    nc = _CACHED_NC
    in_maps = make_in_maps(x, np.asarray(Wq), np.asarray(Wk),
                           np.asarray(Wv), np.asarray(Wo))
    res = None
    for attempt in range(3):
        try:
            res = run_bass_kernel_spmd(nc, in_maps, list(range(N_CORES)),
                                       trace=_trace)
            break
        except Exception:
            if attempt == 2:
                raise
    out = np.zeros((B * S, D), dtype=np.float32)
    for c in range(N_CORES):
        out += res.results[c]["out"].astype(np.float32)
    out += np.asarray(bo, dtype=np.float32)[None, :]
    if _trace:
        kernel._last_result = res
    return out.reshape(B, S, D)


# revision 38
# speedup vs baseline: 1.0979x; 1.0043x over previous
"""Multi-head causal attention (B=2, S=4096, D=1024, H=16) on 8 TRN2 NeuronCores.

Sharding: head-parallel. Core c computes heads 2c, 2c+1 (128 of the 1024
projection columns) for both batches:
  - QKV column-parallel: each core gets Wq/Wk/Wv[:, c*128:(c+1)*128]
  - attention for its 2 heads over all tokens (causal, full score rows,
    streamed in 512-query chunks, keys on PSUM partitions)
  - out-proj row-parallel: partial_out = ctx_c @ Wo[c*128:(c+1)*128, :]
  - host sums the 8 partials and adds bo.

x is transposed on the host (xT = x.reshape(T, D).T) because every matmul
on the PE contracts over the partition axis.

Attention inner loop, per key tile (kt) of each 512-query chunk:
  scores:  PE -> PSUM [128 keys, 2x512 queries (both heads)]; the causal
           mask is a second accumulating matmul adding -30000 to the
           diagonal block's upper triangle (identity stationary).
  exp:     one ACT instruction per kt (strided AP covers both heads'
           valid ranges) -> SBUF ex bf16. ACT is the bottleneck engine
           (~280us of exp); everything else is scheduled around it.
  ctx:     ex is the STATIONARY operand: out[128 queries, 65] +=
           exT @ [v | ones]; the ones column of vA makes the softmax
           denominator land in column 64 (per-query = per-partition).
           Moving operand is 65 columns -> half the PE cost of the
           v-stationary form. All query tiles of a chunk accumulate in
           one PSUM bank per head; only the bank's first matmul carries
           start=True (start zeroes the whole 2KB zero region).
  norm:    as each query tile's accumulation stops (at its diagonal kt),
           DVE reciprocal_approx_fast on the denominator + tensor_scalar
           multiply into a bf16 [128 tok, 128] staging tile.
  cT:      PE transpose (both heads at once) back to dims-major.
  outproj: PE matmul cT-tile @ Wo; DVE evacuation; DMA out.

Scheduling: ctx runs at lag-2 behind scores so its stationary ex is
always ready. Projection windows are split into ~430ns atoms (per-128-
token q/k/v pieces) and chunk tails into two atoms; a priority work
queue drains them only on full-width kts (partial diagonal kts have no
PE slack), preferring pieces needed soonest: next chunk's q pieces,
then near-term k/v pieces, then FIFO.
"""

from contextlib import ExitStack

import numpy as np

import concourse.bass as bass
import concourse.tile as tile
from concourse import bacc, mybir
from concourse.bass_utils import run_bass_kernel_spmd

F32 = mybir.dt.float32
BF16 = mybir.dt.bfloat16
FP8 = mybir.dt.float8e4
P = 128
AF = mybir.ActivationFunctionType

N_CORES = 8
B_FULL, S_FULL, D_FULL, H_FULL = 2, 4096, 1024, 16
DH = 64
CW = 128  # projection columns per core (2 heads * 64)


def build_program(S=S_FULL, B=B_FULL, D=D_FULL):
    """Build the per-core Bass program (same program on all 8 cores)."""
    T = B * S
    KC = D // P            # contraction chunks for the projections
    IC = min(512, S)       # query-chunk width (paired-head layout)
    NQT = IC // P          # query tiles (128) per chunk
    NJ = S // P            # key tiles per batch
    NIC = S // IC          # query chunks per batch
    WN = min(512, T)       # QKV token window

    nc = bacc.Bacc("TRN2", target_bir_lowering=False, debug=False,
                   num_devices=N_CORES)

    xT = nc.dram_tensor("xT", [D, T], BF16, kind="ExternalInput").ap()
    wq = nc.dram_tensor("wq", [P, D // P, CW], BF16, kind="ExternalInput").ap()
    wk = nc.dram_tensor("wk", [P, D // P, CW], BF16, kind="ExternalInput").ap()
    wv = nc.dram_tensor("wv", [P, D // P, CW], BF16, kind="ExternalInput").ap()
    wo = nc.dram_tensor("wo", [CW, D], BF16, kind="ExternalInput").ap()
    maskneg = nc.dram_tensor("maskneg", [P, P], BF16, kind="ExternalInput").ap()
    ident = nc.dram_tensor("ident", [P, P], BF16, kind="ExternalInput").ap()
    out = nc.dram_tensor("out", [T, D], BF16, kind="ExternalOutput").ap()

    with tile.TileContext(nc) as tc, ExitStack() as ctx:
        singles = ctx.enter_context(tc.tile_pool(name="singles", bufs=1))
        qT = singles.tile([P, T], BF16, name="qT")
        kT = singles.tile([P, T], BF16, name="kT")
        vA = singles.tile([P, B * NJ, 130], BF16, name="vA")
        cT = singles.tile([P, T], BF16, name="cT")
        wq_s = singles.tile([P, KC, CW], BF16, name="wq_s")
        wk_s = singles.tile([P, KC, CW], BF16, name="wk_s")
        wv_s = singles.tile([P, KC, CW], BF16, name="wv_s")
        wo_s = singles.tile([CW, D], BF16, name="wo_s")
        maskneg_s = singles.tile([P, P], BF16, name="maskneg_s")
        ident_s = singles.tile([P, P], BF16, name="ident_s")

        nc.sync.dma_start(out=wq_s, in_=wq)
        nc.sync.dma_start(out=wk_s, in_=wk)
        nc.sync.dma_start(out=wv_s, in_=wv)
        nc.vector.memset(vA[:, :, 64:65], 1.0)
        nc.vector.memset(vA[:, :, 129:130], 1.0)

        xw_pool = ctx.enter_context(tc.tile_pool(name="xw_pool", bufs=3))
        # PSUM budget (8 banks): sm 2 + sc 2x2 + ctx 2x1 = 8
        sm_ps = ctx.enter_context(
            tc.tile_pool(name="sm_ps", bufs=2, space=bass.MemorySpace.PSUM))
        sc_ps = ctx.enter_context(
            tc.tile_pool(name="sc_ps", bufs=2, space=bass.MemorySpace.PSUM))
        cx_ps = ctx.enter_context(
            tc.tile_pool(name="cx_ps", bufs=1, space=bass.MemorySpace.PSUM))
        exp_sb = ctx.enter_context(tc.tile_pool(name="exp_sb", bufs=5))
        inv_sb = ctx.enter_context(tc.tile_pool(name="inv_sb", bufs=2))
        cn_sb = ctx.enter_context(tc.tile_pool(name="cn_sb", bufs=6))
        ob_sb = ctx.enter_context(tc.tile_pool(name="ob_sb", bufs=2))

        # --- fine-grained PE filler work queue -------------------------
        # Projection windows and chunk tails are split into ~0.5-2us
        # pieces, drained one per key tile so the PE filler work overlaps
        # ACT's exp (the attention inner loop is ACT-bound).
        fill_q = []          # entries: (kind, key, fn); kind "dma"/"q"
        # keyed by window, "k"/"v" by global token tile, tails (None, 0)

        def push_window(w, first=False):
            cell = {}

            def p_first():
                if first:
                    nc.sync.dma_start(out=wo_s, in_=wo)
                    nc.sync.dma_start(out=maskneg_s, in_=maskneg)
                    nc.sync.dma_start(out=ident_s, in_=ident)
                xw = xw_pool.tile([P, KC, WN], BF16, name="xw", tag="xw")
                cell["xw"] = xw
                for kc in range(KC):
                    nc.sync.dma_start(
                        out=xw[:, kc, :],
                        in_=xT[kc * P:(kc + 1) * P, w * WN:(w + 1) * WN])

            def p_q(st):
                xw = cell["xw"]
                t0 = w * WN + st * P
                q_ps = sm_ps.tile([P, P], F32, name="q_ps", tag="sm")
                for kc in range(KC):
                    nc.tensor.matmul(q_ps, wq_s[:, kc, :],
                                     xw[:, kc, st * P:(st + 1) * P],
                                     start=(kc == 0), stop=(kc == KC - 1))
                nc.vector.tensor_copy(qT[:, t0:t0 + P], q_ps)

            def p_k(st):
                xw = cell["xw"]
                t0 = w * WN + st * P
                k_ps = sm_ps.tile([P, P], F32, name="k_ps", tag="sm")
                for kc in range(KC):
                    nc.tensor.matmul(k_ps, wk_s[:, kc, :],
                                     xw[:, kc, st * P:(st + 1) * P],
                                     start=(kc == 0), stop=(kc == KC - 1))
                nc.vector.tensor_copy(kT[:, t0:t0 + P], k_ps)

            def p_v(st):
                xw = cell["xw"]
                jt = (w * WN) // P + st  # global token tile
                vp = sm_ps.tile([P, CW], F32, name="vp", tag="sm")
                for kc in range(KC):
                    nc.tensor.matmul(vp, xw[:, kc, st * P:(st + 1) * P],
                                     wv_s[:, kc, :],
                                     start=(kc == 0), stop=(kc == KC - 1))
                dst = vA[:, jt, 0:129]
                dst = bass.AP(tensor=dst.tensor, offset=dst.offset,
                              ap=[dst.ap[0], [65, 2], [1, 64]])
                srcap = bass.AP(tensor=vp.tensor, offset=vp.offset,
                                ap=[vp.ap[0], [64, 2], [1, 64]])
                nc.vector.tensor_copy(dst, srcap)

            def fused(fns):
                def run():
                    for f in fns:
                        f()
                return run

            nst = WN // P
            t0 = (w * WN) // P
            fill_q.append(("dma", w, p_first))
            for st in range(nst):
                fill_q.append(("q", w, lambda st=st: p_q(st)))
            for st in range(nst):
                fill_q.append(("k", t0 + st, lambda st=st: p_k(st)))
                fill_q.append(("v", t0 + st, lambda st=st: p_v(st)))

        def _run(ent):
            kind, key, fn = ent
            if kind in ("q", "k", "v"):
                w = key if kind == "q" else (key * P) // WN
                for idx, e2 in enumerate(fill_q):
                    if e2[0] == "dma" and e2[1] == w:
                        fill_q.pop(idx)
                        e2[2]()
                        break
            fn()

        horizon = {"win": 0, "tile": 0}

        def drain_one():
            if not fill_q:
                return False
            pick = None
            for idx, ent in enumerate(fill_q):
                if ent[0] in ("dma", "q") and ent[1] <= horizon["win"]:
                    pick = idx
                    break
            if pick is None:
                for idx, ent in enumerate(fill_q):
                    if ent[0] in ("k", "v") and ent[1] <= horizon["tile"]:
                        pick = idx
                        break
            _run(fill_q.pop(pick if pick is not None else 0))
            return True

        def need(pred):
            while True:
                for idx, ent in enumerate(fill_q):
                    if pred(ent):
                        _run(fill_q.pop(idx))
                        break
                else:
                    return

        attn_pend = {"fns": []}

        def emit_attn_chunk(b, icn):
            gi0 = b * S + icn * IC  # global query start
            njt = (icn + 1) * NQT   # key tiles for this chunk
            # per-head ctx accumulators [128 queries, qt, 65]; stride 80
            # keeps each qt slice 64B-aligned in its bank
            cxs = [cx_ps.tile([P, NQT, 80], F32, name=f"cx{h}", tag=f"cx{h}")
                   for h in range(2)]
            invs = [inv_sb.tile([P, NQT], F32, name="inv", tag=f"inv{h}",
                                bufs=3) for h in range(2)]

            def make_tails(l, cn):
                cell = {}
                endgame = (b == B - 1 and icn == NIC - 1)

                def tail_a():
                    s0 = gi0 + l * P
                    tr = sm_ps.tile([P, P], BF16, name="tr", tag="sm")
                    nc.tensor.transpose(tr, cn, ident_s)
                    nc.vector.tensor_copy(cT[:, s0:s0 + P], tr)
                    ob = ob_sb.tile([P, D], BF16, name="ob", tag="ob")
                    cell["ob"] = ob
                    op = sm_ps.tile([P, 512], F32, name="op", tag="sm")
                    nc.tensor.matmul(op, cT[:, s0:s0 + P], wo_s[:, 0:512],
                                     start=True, stop=True)
                    if endgame:
                        # ACT is idle after the last exp: split the final
                        # psum evacuations across both engines
                        nc.scalar.copy(ob[:, 0:512], op)
                    else:
                        nc.vector.tensor_copy(ob[:, 0:512], op)

                def tail_b():
                    s0 = gi0 + l * P
                    ob = cell["ob"]
                    op = sm_ps.tile([P, 512], F32, name="op", tag="sm")
                    nc.tensor.matmul(op, cT[:, s0:s0 + P], wo_s[:, 512:1024],
                                     start=True, stop=True)
                    nc.vector.tensor_copy(ob[:, 512:1024], op)
                    nc.sync.dma_start(out=out[s0:s0 + P, :], in_=ob)
                return tail_a, tail_b

            def emit_ctx_and_norm(jt, ex):
                # ctx matmuls for key tile jt, all query tiles >= jt.
                # start=True zeroes the whole 2KB psum bank (the "zero
                # region"), so only the bank's FIRST matmul sets it; the
                # other qtiles' first writes overwrite their still
                # pending-zero bytes and later writes accumulate.
                for h in range(2):
                    for l in range(NQT):
                        qt_g = icn * NQT + l  # global query tile
                        if qt_g < jt:
                            continue
                        nc.tensor.matmul(
                            cxs[h][:, l, 0:65],
                            ex[:, h * IC + l * P:h * IC + (l + 1) * P],
                            vA[:, b * NJ + jt, h * 65:(h + 1) * 65],
                            start=(jt == 0 and l == 0), stop=(jt == qt_g),
                            skip_group_check=True)
                ld = jt - icn * NQT
                if ld >= 0:
                    # qtile ld just finished accumulating: normalize now so
                    # the ctx banks free early; transpose+outproj deferred
                    cn = cn_sb.tile([P, P], BF16, name="cn", tag="cn",
                                    bufs=10)
                    for h in range(2):
                        nc.vector.reciprocal_approx_fast(
                            invs[h][:, ld:ld + 1], cxs[h][:, ld, 64:65])
                        nc.vector.tensor_scalar_mul(
                            cn[:, h * 64:(h + 1) * 64],
                            cxs[h][:, ld, 0:64], invs[h][:, ld:ld + 1])
                    ta, tb = make_tails(ld, cn)
                    fill_q.append(("tail", 0, ta))
                    fill_q.append(("tail", 0, tb))

            for jt in range(njt):
                il0 = max(0, jt * P - icn * IC)
                gj0 = b * S + jt * P
                diag = jt * P >= icn * IC
                gt = gj0 // P
                need(lambda e: e[0] in ("k", "v") and e[1] <= gt)
                horizon["tile"] = gt + 4
                sc = sc_ps.tile([P, 2 * IC], F32, name="sc", tag="sc")
                for h in range(2):
                    hp = h * 64
                    nc.tensor.matmul(
                        sc[:, h * IC + il0:(h + 1) * IC],
                        kT[hp:hp + 64, gj0:gj0 + P],
                        qT[hp:hp + 64, gi0 + il0:gi0 + IC],
                        start=True, stop=not diag)
                    if diag:
                        # causal mask: add -30000 to the strict upper
                        # triangle of the diagonal block on the PE
                        nc.tensor.matmul(
                            sc[:, h * IC + il0:h * IC + il0 + P],
                            ident_s, maskneg_s,
                            start=False, stop=True)
                ex = exp_sb.tile([P, 2 * IC], BF16, name="ex", tag="ex")
                exin = bass.AP(tensor=sc.tensor, offset=sc.offset + il0,
                               ap=[sc.ap[0], [IC, 2], [1, IC - il0]])
                exout = bass.AP(tensor=ex.tensor, offset=ex.offset + il0,
                                ap=[ex.ap[0], [IC, 2], [1, IC - il0]])
                nc.scalar.activation(exout, exin, AF.Exp, scale=0.125)
                # lag-2 software pipeline ACROSS chunks: ctx of key tile
                # jt-2 issues after scores of jt, so its stationary ex has
                # been ready for two exp-latencies and the PE never stalls
                if len(attn_pend["fns"]) >= 2:
                    attn_pend["fns"].pop(0)()
                # drain fillers only on full-width kts: partial diagonal
                # kts have short exps and no PE slack to hide filler work
                if il0 == 0:
                    drain_one()
                    if njt >= 16 and len(fill_q) > 40:
                        drain_one()
                elif any(e[0] in ("dma", "q") and e[1] <= horizon["win"]
                         for e in fill_q):
                    # pre-clear the next chunk's query-window pieces even
                    # on diag kts: a small gap here beats a chunk-start
                    # burst that stalls the next chunk's first exp
                    drain_one()
                if len(fill_q) < 16:
                    push_upto(state["pushed"] + 1)
                attn_pend["fns"].append(
                    lambda jt=jt, ex=ex: emit_ctx_and_norm(jt, ex))

        # Emission: window w covers tokens [w*WN,(w+1)*WN); chunk (b,
        # icn) needs windows covering tokens < b*S + (icn+1)*IC pushed
        # and drained before its scores; two extra windows of lookahead
        # keep the DMA off the critical path.
        nwin = T // WN
        state = {"pushed": 0}

        def push_upto(upto):
            while state["pushed"] < min(upto, nwin):
                push_window(state["pushed"], first=(state["pushed"] == 0))
                state["pushed"] += 1

        for b in range(B):
            for icn in range(NIC):
                wq_win = (b * S + icn * IC) // WN
                push_upto(wq_win + 3)
                need(lambda e: e[0] in ("dma", "q") and e[1] <= wq_win)
                horizon["win"] = min(wq_win + 1, nwin - 1)
                if icn == NIC - 1:
                    horizon["win"] = min(((b + 1) * S) // WN, nwin - 1)
                emit_attn_chunk(b, icn)
        for fn in attn_pend["fns"]:
            fn()
        attn_pend["fns"] = []
        while drain_one():
            pass

    nc.compile()
    return nc


def _warrange(w, bf16):
    # [D, CW] -> [P, D//P, CW] contiguous (the SBUF layout, so the DMA is
    # a single contiguous copy instead of 256B strided pieces)
    D, CW_ = w.shape
    return np.ascontiguousarray(
        w.reshape(D // P, P, CW_).transpose(1, 0, 2)).astype(bf16)


def make_in_maps(x, Wq, Wk, Wv, Wo):
    import ml_dtypes
    bf16 = ml_dtypes.bfloat16
    B, S, D = x.shape
    xT = np.ascontiguousarray(x.reshape(B * S, D).T).astype(bf16)
    maskneg = np.tril(np.full((P, P), -30000.0, dtype=np.float32),
                      -1).astype(bf16)
    ident = np.eye(P, dtype=bf16)
    in_maps = []
    for c in range(N_CORES):
        cs = slice(c * CW, (c + 1) * CW)
        in_maps.append({
            "xT": xT,
            "wq": _warrange(Wq[:, cs], bf16),
            "wk": _warrange(Wk[:, cs], bf16),
            "wv": _warrange(Wv[:, cs], bf16),
            "wo": np.ascontiguousarray(Wo[cs, :]).astype(bf16),
            "maskneg": maskneg,
            "ident": ident,
        })
    return in_maps


_CACHED_NC = None


def kernel(x, Wq, Wk, Wv, Wo, bo, _trace=False):
    global _CACHED_NC
    x = np.asarray(x, dtype=np.float32)
    B, S, D = x.shape
    if _CACHED_NC is None:
        _CACHED_NC = build_program(S=S, B=B, D=D)
    nc = _CACHED_NC
    in_maps = make_in_maps(x, np.asarray(Wq), np.asarray(Wk),
                           np.asarray(Wv), np.asarray(Wo))
    res = None
    for attempt in range(3):
        try:
            res = run_bass_kernel_spmd(nc, in_maps, list(range(N_CORES)),
                                       trace=_trace)
            break
        except Exception:
            if attempt == 2:
                raise
    out = np.zeros((B * S, D), dtype=np.float32)
    for c in range(N_CORES):
        out += res.results[c]["out"].astype(np.float32)
    out += np.asarray(bo, dtype=np.float32)[None, :]
    if _trace:
        kernel._last_result = res
    return out.reshape(B, S, D)
